# revision 7
# baseline (speedup 1.0000x reference)
"""ConvLSTM (peephole) Trainium2 Bass kernel.

Data-parallel over batch: 8 cores, one batch element each; conv/peephole
weights replicated. The time recurrence runs on-chip per core.

Conv3x3(SAME) over concat(x, h) is computed as 12 accumulating matmuls per
(output-channel chunk, spatial chunk): activations are stored in SBUF in a
zero-padded 66x66 spatial layout so each of the 9 kernel taps is a pure
access-pattern offset, and a 3-block "dy-stack" (x/h replicated at row
shifts -66/0/+66 across 480 partition rows -> 4 K-tiles of 128/128/128/96)
packs the 1440-row contraction into ceil(1440/128)=12 full matmuls.
"""

import numpy as np
import ml_dtypes

IN_CH, HID = 64, 96
B, T, H, W_SP = 8, 16, 64, 64
HP = WP = 66
PADN = HP * WP          # 4356
INT_N = H * W_SP        # 4096
NCHUNK = 8              # spatial chunks per timestep
ROWS_PER_CHUNK = H // NCHUNK   # 8
CHW = ROWS_PER_CHUNK * W_SP    # 512
SPAN = ROWS_PER_CHUNK * WP     # 528 padded cols per chunk
TILE_K = [128, 128, 128, 96]   # partition rows of the 4 dy-stack tiles

_CACHE = {}


def _build_nc():
    import concourse.bass as bass
    import concourse.tile as tile
    from concourse import mybir, bacc
    from contextlib import ExitStack

    f32 = mybir.dt.float32
    bf16 = mybir.dt.bfloat16
    AF = mybir.ActivationFunctionType
    ALU = mybir.AluOpType

    nc = bacc.Bacc("TRN2", target_bir_lowering=False, debug=False)

    xp = nc.dram_tensor("xp", [T, IN_CH, PADN], bf16, kind="ExternalInput").ap()
    wt = nc.dram_tensor("wt", [36, 128, 128], bf16, kind="ExternalInput").ap()
    pp = nc.dram_tensor("pp", [HID, 7], f32, kind="ExternalInput").ap()
    y = nc.dram_tensor("y", [T, HID, INT_N], f32, kind="ExternalOutput").ap()

    with tile.TileContext(nc) as tc, ExitStack() as ctx:
        const_pool = ctx.enter_context(tc.tile_pool(name="const", bufs=1))
        tset_pool = ctx.enter_context(tc.tile_pool(name="tset", bufs=1))
        tmp_pool = ctx.enter_context(tc.tile_pool(name="tmp", bufs=3))
        hout_pool = ctx.enter_context(tc.tile_pool(name="hout", bufs=4))
        psum_pool = ctx.enter_context(tc.tile_pool(name="psum", bufs=2, space="PSUM"))

        # persistent tensors
        wt_sb = const_pool.tile([128, 36 * 128], bf16, tag="wt_sb", name="wt_sb")
        pp_sb = const_pool.tile([HID, 7], f32, tag="pp_sb", name="pp_sb")
        c_sb = const_pool.tile([HID, INT_N], f32, tag="c_sb", name="c_sb")
        tsets = [
            [tset_pool.tile([TILE_K[i], PADN], bf16, tag=f"T{p}_{i}", name=f"T{p}_{i}") for i in range(4)]
            for p in range(2)
        ]

        # one-time loads
        nc.sync.dma_start(
            wt_sb[:].rearrange("k (i m) -> k i m", m=128), wt.rearrange("i k m -> k i m")
        )
        nc.sync.dma_start(pp_sb[:], pp[:])

        # zero-init T sets (borders stay zero forever) and cell state
        eng = [nc.vector, nc.gpsimd]
        for p in range(2):
            for i in range(4):
                eng[(p * 4 + i) % 2].memset(tsets[p][i][:], 0.0)
        nc.vector.memset(c_sb[:], 0.0)

        w_ci, w_cf, w_co = pp_sb[:, 0:1], pp_sb[:, 1:2], pp_sb[:, 2:3]
        b_i, b_f, b_g, b_o = (pp_sb[:, 3 + k : 4 + k] for k in range(4))

        for t in range(T):
            src = tsets[t % 2]
            dst = tsets[(t + 1) % 2]

            # load x_t into the three dy blocks of the source T set
            nc.sync.dma_start(src[0][0:64, 66:PADN], xp[t, :, 0 : PADN - 66])
            nc.sync.dma_start(src[1][32:96, 0:PADN], xp[t, :, 0:PADN])
            nc.sync.dma_start(src[2][64:128, 0 : PADN - 66], xp[t, :, 66:PADN])

            src3d = [src[i][:].rearrange("k (h w) -> k h w", w=WP) for i in range(4)]

            for s in range(NCHUNK):
                ps = [psum_pool.tile([128, CHW], f32, tag=f"p{co}", name=f"p{co}") for co in range(3)]
                r0 = ROWS_PER_CHUNK * s
                for co in range(3):
                    for dxi in range(3):
                        for ti in range(4):
                            k = TILE_K[ti]
                            idx = (co * 3 + dxi) * 4 + ti
                            nc.tensor.matmul(
                                ps[co][:, :],
                                wt_sb[0:k, 128 * idx : 128 * (idx + 1)],
                                src3d[ti][0:k, r0 + 1 : r0 + 1 + ROWS_PER_CHUNK, dxi : dxi + 64],
                                start=(dxi == 0 and ti == 0),
                                stop=(dxi == 2 and ti == 3),
                            )
                p0, p1, p2 = ps

                # pointwise LSTM cell on this 512-position chunk
                cs = c_sb[:, CHW * s : CHW * (s + 1)]
                ti_ = tmp_pool.tile([HID, CHW], f32, tag="ti", name="ti")
                tf_ = tmp_pool.tile([HID, CHW], f32, tag="tf", name="tf")
                g_ = tmp_pool.tile([HID, CHW], f32, tag="g", name="g")
                to_ = tmp_pool.tile([HID, CHW], f32, tag="to", name="to")
                th_ = tmp_pool.tile([HID, CHW], f32, tag="th", name="th")
                ho = hout_pool.tile([HID, CHW], f32, tag="ho", name="ho")

                # PSUM piece -> hid range map (gate channels permuted in lhsT
                # so every engine AP stays within an aligned partition quadrant)
                nc.vector.scalar_tensor_tensor(
                    ti_[:], cs, w_ci, p0[0:96, :], ALU.mult, ALU.add
                )
                nc.vector.scalar_tensor_tensor(
                    tf_[64:96, :], cs[64:96, :], w_cf[64:96, :], p0[96:128, :],
                    ALU.mult, ALU.add,
                )
                nc.vector.scalar_tensor_tensor(
                    tf_[0:64, :], cs[0:64, :], w_cf[0:64, :], p1[0:64, :],
                    ALU.mult, ALU.add,
                )
                nc.scalar.activation(ti_[:], ti_[:], AF.Sigmoid, bias=b_i)
                nc.scalar.activation(tf_[:], tf_[:], AF.Sigmoid, bias=b_f)
                nc.scalar.activation(g_[0:64, :], p1[64:128, :], AF.Tanh, bias=b_g[0:64, :])
                nc.scalar.activation(g_[64:96, :], p2[0:32, :], AF.Tanh, bias=b_g[64:96, :])
                nc.vector.tensor_mul(g_[:], g_[:], ti_[:])      # i * tanh(gg)
                nc.vector.tensor_mul(tf_[:], tf_[:], cs)        # f * c_old
                nc.vector.tensor_add(cs, tf_[:], g_[:])         # c_new
                nc.vector.scalar_tensor_tensor(
                    to_[64:96, :], cs[64:96, :], w_co[64:96, :], p2[32:64, :],
                    ALU.mult, ALU.add,
                )
                nc.vector.scalar_tensor_tensor(
                    to_[0:64, :], cs[0:64, :], w_co[0:64, :], p2[64:128, :],
                    ALU.mult, ALU.add,
                )
                nc.scalar.activation(to_[:], to_[:], AF.Sigmoid, bias=b_o)
                nc.scalar.activation(th_[:], cs, AF.Tanh)
                nc.vector.tensor_mul(ho[:], to_[:], th_[:])     # h = o * tanh(c)

                nc.sync.dma_start(y[t, :, CHW * s : CHW * (s + 1)], ho[:])

                # write h (cast bf16) into block-2 rows of next T set, then
                # replicate to the other dy blocks
                d3 = dst[3][:].rearrange("k (h w) -> k h w", w=WP)
                nc.scalar.activation(
                    d3[:, r0 : r0 + ROWS_PER_CHUNK, 1:65],
                    ho[:].rearrange("k (h w) -> k h w", w=64),
                    AF.Copy,
                )
                a, b_ = WP * ROWS_PER_CHUNK * s, WP * ROWS_PER_CHUNK * s + SPAN
                nc.sync.dma_start(dst[0][64:128, a + 132 : b_ + 132], dst[3][0:64, a:b_])
                nc.sync.dma_start(dst[1][0:32, a + 132 : b_ + 132], dst[3][64:96, a:b_])
                nc.sync.dma_start(dst[1][96:128, a + 66 : b_ + 66], dst[3][0:32, a:b_])
                nc.sync.dma_start(dst[2][0:64, a + 66 : b_ + 66], dst[3][32:96, a:b_])

    nc.compile()
    return nc


def _host_inputs(x_seq, W, b, w_ci, w_cf, w_co):
    bf16 = ml_dtypes.bfloat16
    # padded, pre-cast x: [B, T, IN_CH, PADN]
    xp = np.zeros((B, T, IN_CH, HP, WP), np.float32)
    xp[:, :, :, 1:65, 1:65] = x_seq
    xp = xp.reshape(B, T, IN_CH, PADN).astype(bf16)

    # PSUM row q -> W output row (per-gate hid permutation keeping every
    # pointwise engine AP inside an aligned partition quadrant)
    q = np.arange(384)
    perm = q.copy()
    perm[(96 <= q) & (q < 128)] = q[(96 <= q) & (q < 128)] + 64
    perm[(128 <= q) & (q < 192)] = q[(128 <= q) & (q < 192)] - 32
    perm[(288 <= q) & (q < 320)] = q[(288 <= q) & (q < 320)] + 64
    perm[(320 <= q) & (q < 384)] = q[(320 <= q) & (q < 384)] - 32

    # lhsT tiles: [(co*3+dxi)*4+tile, 128, 128]
    lhsT = np.zeros((3, 3, 4, 128, 128), np.float32)
    for ti in range(4):
        kt = TILE_K[ti]
        sr = 128 * ti + np.arange(kt)
        blk = sr // 160          # dy block: ky index
        cch = sr % 160           # channel within concat(x, h)
        for co in range(3):
            for dxi in range(3):
                wrows = perm[co * 128 : (co + 1) * 128]
                lhsT[co, dxi, ti, :kt, :] = W[wrows][:, cch, blk, dxi].T
    wt = lhsT.reshape(36, 128, 128).astype(bf16)

    pp = np.stack(
        [
            w_ci[:, 0, 0], w_cf[:, 0, 0], w_co[:, 0, 0],
            b[0:96], b[96:192], b[192:288], b[288:384],
        ],
        axis=1,
    ).astype(np.float32)
    return xp, wt, pp


def kernel(x_seq, W, b, w_ci, w_cf, w_co):
    from concourse import bass_utils

    if "nc" not in _CACHE:
        _CACHE["nc"] = _build_nc()
    nc = _CACHE["nc"]

    xp, wt, pp = _host_inputs(
        np.asarray(x_seq, np.float32), np.asarray(W, np.float32),
        np.asarray(b, np.float32), np.asarray(w_ci, np.float32),
        np.asarray(w_cf, np.float32), np.asarray(w_co, np.float32),
    )
    in_maps = [{"xp": xp[i], "wt": wt, "pp": pp} for i in range(B)]

    last = None
    for _ in range(3):  # retry: first exec after a wedged device can flake
        try:
            res = bass_utils.run_bass_kernel_spmd(nc, in_maps, list(range(B)))
            break
        except Exception as e:  # noqa: BLE001
            last = e
    else:
        raise last

    out = np.stack(
        [res.results[i]["y"].reshape(T, HID, H, W_SP) for i in range(B)], axis=0
    )
    return out.astype(np.float32)


# revision 10
# speedup vs baseline: 1.0181x; 1.0181x over previous
"""ConvLSTM (peephole) Trainium2 Bass kernel.

Data-parallel over batch: 8 cores, one batch element each; conv/peephole
weights replicated. The time recurrence runs on-chip per core.

Conv3x3(SAME) over concat(x, h) is computed as 12 accumulating matmuls per
(output-channel chunk, spatial chunk): activations are stored in SBUF in a
zero-padded 66x66 spatial layout so each of the 9 kernel taps is a pure
access-pattern offset, and a 3-block "dy-stack" (x/h replicated at row
shifts -66/0/+66 across 480 partition rows -> 4 K-tiles of 128/128/128/96)
packs the 1440-row contraction into ceil(1440/128)=12 full matmuls.
"""

import numpy as np
import ml_dtypes

IN_CH, HID = 64, 96
B, T, H, W_SP = 8, 16, 64, 64
HP = WP = 66
PADN = HP * WP          # 4356
INT_N = H * W_SP        # 4096
NCHUNK = 8              # spatial chunks per timestep
ROWS_PER_CHUNK = H // NCHUNK   # 8
CHW = ROWS_PER_CHUNK * W_SP    # 512
SPAN = ROWS_PER_CHUNK * WP     # 528 padded cols per chunk
TILE_K = [128, 128, 128, 96]   # partition rows of the 4 dy-stack tiles

_CACHE = {}


def _build_nc():
    import concourse.bass as bass
    import concourse.tile as tile
    from concourse import mybir, bacc
    from contextlib import ExitStack

    f32 = mybir.dt.float32
    bf16 = mybir.dt.bfloat16
    AF = mybir.ActivationFunctionType
    ALU = mybir.AluOpType

    nc = bacc.Bacc("TRN2", target_bir_lowering=False, debug=False)

    xp = nc.dram_tensor("xp", [T, IN_CH, PADN], bf16, kind="ExternalInput").ap()
    wt = nc.dram_tensor("wt", [36, 128, 128], bf16, kind="ExternalInput").ap()
    pp = nc.dram_tensor("pp", [HID, 7], f32, kind="ExternalInput").ap()
    y = nc.dram_tensor("y", [T, HID, INT_N], f32, kind="ExternalOutput").ap()

    with tile.TileContext(nc) as tc, ExitStack() as ctx:
        const_pool = ctx.enter_context(tc.tile_pool(name="const", bufs=1))
        tset_pool = ctx.enter_context(tc.tile_pool(name="tset", bufs=1))
        tmp_pool = ctx.enter_context(tc.tile_pool(name="tmp", bufs=3))
        hout_pool = ctx.enter_context(tc.tile_pool(name="hout", bufs=4))
        psum_pool = ctx.enter_context(tc.tile_pool(name="psum", bufs=2, space="PSUM"))

        # persistent tensors
        wt_sb = const_pool.tile([128, 36 * 128], bf16, tag="wt_sb", name="wt_sb")
        pp_sb = const_pool.tile([HID, 7], f32, tag="pp_sb", name="pp_sb")
        c_sb = const_pool.tile([HID, INT_N], f32, tag="c_sb", name="c_sb")
        tsets = [
            [tset_pool.tile([TILE_K[i], PADN], bf16, tag=f"T{p}_{i}", name=f"T{p}_{i}") for i in range(4)]
            for p in range(2)
        ]

        # one-time loads
        nc.sync.dma_start(
            wt_sb[:].rearrange("k (i m) -> k i m", m=128), wt.rearrange("i k m -> k i m")
        )
        nc.sync.dma_start(pp_sb[:], pp[:])

        # zero-init: only regions not overwritten by the x DMAs / h casts /
        # h replication, so the first x DMA isn't serialized behind full-tile
        # memsets. Parity-0 h rows must be fully zero (t=0 reads them).
        Tz = tsets[0]
        nc.vector.memset(Tz[0][64:128, :], 0.0)
        nc.gpsimd.memset(Tz[1][0:32, :], 0.0)
        nc.vector.memset(Tz[1][96:128, :], 0.0)
        nc.gpsimd.memset(Tz[2][0:64, :], 0.0)
        nc.vector.memset(Tz[3][0:96, :], 0.0)
        To = tsets[1]
        nc.gpsimd.memset(To[0][64:128, 0:132], 0.0)
        nc.gpsimd.memset(To[1][0:32, 0:132], 0.0)
        nc.gpsimd.memset(To[1][96:128, 0:66], 0.0)
        nc.gpsimd.memset(To[2][0:64, 0:66], 0.0)
        nc.gpsimd.memset(To[1][96:128, PADN - 66 : PADN], 0.0)
        nc.gpsimd.memset(To[2][0:64, PADN - 66 : PADN], 0.0)
        nc.vector.memset(To[3][0:96, PADN - 132 : PADN], 0.0)
        d3o = To[3][:].rearrange("k (h w) -> k h w", w=WP)
        nc.vector.memset(d3o[:, 0:64, 0:1], 0.0)   # T3 pad cols never cast-written
        nc.vector.memset(d3o[:, 0:64, 65:66], 0.0)
        for p in range(2):  # x-row edge strips the x DMA never covers
            nc.vector.memset(tsets[p][0][0:64, 0:66], 0.0)
            nc.vector.memset(tsets[p][2][64:128, PADN - 66 : PADN], 0.0)
        nc.vector.memset(c_sb[:], 0.0)

        w_ci, w_cf, w_co = pp_sb[:, 0:1], pp_sb[:, 1:2], pp_sb[:, 2:3]
        b_i, b_f, b_g, b_o = (pp_sb[:, 3 + k : 4 + k] for k in range(4))

        for t in range(T):
            src = tsets[t % 2]
            dst = tsets[(t + 1) % 2]

            # load x_t into the three dy blocks of the source T set
            nc.sync.dma_start(src[0][0:64, 66:PADN], xp[t, :, 0 : PADN - 66])
            nc.sync.dma_start(src[1][32:96, 0:PADN], xp[t, :, 0:PADN])
            nc.sync.dma_start(src[2][64:128, 0 : PADN - 66], xp[t, :, 66:PADN])

            src3d = [src[i][:].rearrange("k (h w) -> k h w", w=WP) for i in range(4)]

            for s in range(NCHUNK):
                ps = [psum_pool.tile([128, CHW], f32, tag=f"p{co}", name=f"p{co}") for co in range(3)]
                r0 = ROWS_PER_CHUNK * s
                last_ti = 2 if t == 0 else 3  # tile 3 is pure-h: all zero at t=0
                for co in range(3):
                    for dxi in range(3):
                        for ti in range(4):
                            if t == 0 and ti == 3:
                                continue
                            k = TILE_K[ti]
                            idx = (co * 3 + dxi) * 4 + ti
                            nc.tensor.matmul(
                                ps[co][:, :],
                                wt_sb[0:k, 128 * idx : 128 * (idx + 1)],
                                src3d[ti][0:k, r0 + 1 : r0 + 1 + ROWS_PER_CHUNK, dxi : dxi + 64],
                                start=(dxi == 0 and ti == 0),
                                stop=(dxi == 2 and ti == last_ti),
                            )
                p0, p1, p2 = ps

                # pointwise LSTM cell on this 512-position chunk
                cs = c_sb[:, CHW * s : CHW * (s + 1)]
                ti_ = tmp_pool.tile([HID, CHW], f32, tag="ti", name="ti")
                tf_ = tmp_pool.tile([HID, CHW], f32, tag="tf", name="tf")
                g_ = tmp_pool.tile([HID, CHW], f32, tag="g", name="g")
                to_ = tmp_pool.tile([HID, CHW], f32, tag="to", name="to")
                th_ = tmp_pool.tile([HID, CHW], f32, tag="th", name="th")
                ho = hout_pool.tile([HID, CHW], f32, tag="ho", name="ho")

                # PSUM piece -> hid range map (gate channels permuted in lhsT
                # so every engine AP stays within an aligned partition quadrant)
                nc.vector.scalar_tensor_tensor(
                    ti_[:], cs, w_ci, p0[0:96, :], ALU.mult, ALU.add
                )
                nc.vector.scalar_tensor_tensor(
                    tf_[64:96, :], cs[64:96, :], w_cf[64:96, :], p0[96:128, :],
                    ALU.mult, ALU.add,
                )
                nc.vector.scalar_tensor_tensor(
                    tf_[0:64, :], cs[0:64, :], w_cf[0:64, :], p1[0:64, :],
                    ALU.mult, ALU.add,
                )
                nc.scalar.activation(ti_[:], ti_[:], AF.Sigmoid, bias=b_i)
                nc.scalar.activation(tf_[:], tf_[:], AF.Sigmoid, bias=b_f)
                nc.scalar.activation(g_[0:64, :], p1[64:128, :], AF.Tanh, bias=b_g[0:64, :])
                nc.scalar.activation(g_[64:96, :], p2[0:32, :], AF.Tanh, bias=b_g[64:96, :])
                nc.vector.tensor_mul(g_[:], g_[:], ti_[:])      # i * tanh(gg)
                nc.vector.tensor_mul(tf_[:], tf_[:], cs)        # f * c_old
                nc.vector.tensor_add(cs, tf_[:], g_[:])         # c_new
                nc.vector.scalar_tensor_tensor(
                    to_[64:96, :], cs[64:96, :], w_co[64:96, :], p2[32:64, :],
                    ALU.mult, ALU.add,
                )
                nc.vector.scalar_tensor_tensor(
                    to_[0:64, :], cs[0:64, :], w_co[0:64, :], p2[64:128, :],
                    ALU.mult, ALU.add,
                )
                nc.scalar.activation(to_[:], to_[:], AF.Sigmoid, bias=b_o)
                nc.scalar.activation(th_[:], cs, AF.Tanh)
                nc.vector.tensor_mul(ho[:], to_[:], th_[:])     # h = o * tanh(c)

                nc.sync.dma_start(y[t, :, CHW * s : CHW * (s + 1)], ho[:])

                if t + 1 < T:
                    # write h (cast bf16) into block-2 rows of next T set,
                    # then replicate to the other dy blocks
                    d3 = dst[3][:].rearrange("k (h w) -> k h w", w=WP)
                    nc.scalar.activation(
                        d3[:, r0 : r0 + ROWS_PER_CHUNK, 1:65],
                        ho[:].rearrange("k (h w) -> k h w", w=64),
                        AF.Copy,
                    )
                    a, b_ = WP * ROWS_PER_CHUNK * s, WP * ROWS_PER_CHUNK * s + SPAN
                    nc.sync.dma_start(dst[0][64:128, a + 132 : b_ + 132], dst[3][0:64, a:b_])
                    nc.sync.dma_start(dst[1][0:32, a + 132 : b_ + 132], dst[3][64:96, a:b_])
                    nc.sync.dma_start(dst[1][96:128, a + 66 : b_ + 66], dst[3][0:32, a:b_])
                    nc.sync.dma_start(dst[2][0:64, a + 66 : b_ + 66], dst[3][32:96, a:b_])

    nc.compile()
    return nc


def _host_inputs(x_seq, W, b, w_ci, w_cf, w_co):
    bf16 = ml_dtypes.bfloat16
    # padded, pre-cast x: [B, T, IN_CH, PADN]
    xp = np.zeros((B, T, IN_CH, HP, WP), np.float32)
    xp[:, :, :, 1:65, 1:65] = x_seq
    xp = xp.reshape(B, T, IN_CH, PADN).astype(bf16)

    # PSUM row q -> W output row (per-gate hid permutation keeping every
    # pointwise engine AP inside an aligned partition quadrant)
    q = np.arange(384)
    perm = q.copy()
    perm[(96 <= q) & (q < 128)] = q[(96 <= q) & (q < 128)] + 64
    perm[(128 <= q) & (q < 192)] = q[(128 <= q) & (q < 192)] - 32
    perm[(288 <= q) & (q < 320)] = q[(288 <= q) & (q < 320)] + 64
    perm[(320 <= q) & (q < 384)] = q[(320 <= q) & (q < 384)] - 32

    # lhsT tiles: [(co*3+dxi)*4+tile, 128, 128]
    lhsT = np.zeros((3, 3, 4, 128, 128), np.float32)
    for ti in range(4):
        kt = TILE_K[ti]
        sr = 128 * ti + np.arange(kt)
        blk = sr // 160          # dy block: ky index
        cch = sr % 160           # channel within concat(x, h)
        for co in range(3):
            for dxi in range(3):
                wrows = perm[co * 128 : (co + 1) * 128]
                lhsT[co, dxi, ti, :kt, :] = W[wrows][:, cch, blk, dxi].T
    wt = lhsT.reshape(36, 128, 128).astype(bf16)

    pp = np.stack(
        [
            w_ci[:, 0, 0], w_cf[:, 0, 0], w_co[:, 0, 0],
            b[0:96], b[96:192], b[192:288], b[288:384],
        ],
        axis=1,
    ).astype(np.float32)
    return xp, wt, pp


def kernel(x_seq, W, b, w_ci, w_cf, w_co):
    from concourse import bass_utils

    if "nc" not in _CACHE:
        _CACHE["nc"] = _build_nc()
    nc = _CACHE["nc"]

    xp, wt, pp = _host_inputs(
        np.asarray(x_seq, np.float32), np.asarray(W, np.float32),
        np.asarray(b, np.float32), np.asarray(w_ci, np.float32),
        np.asarray(w_cf, np.float32), np.asarray(w_co, np.float32),
    )
    in_maps = [{"xp": xp[i], "wt": wt, "pp": pp} for i in range(B)]

    last = None
    for _ in range(3):  # retry: first exec after a wedged device can flake
        try:
            res = bass_utils.run_bass_kernel_spmd(nc, in_maps, list(range(B)))
            break
        except Exception as e:  # noqa: BLE001
            last = e
    else:
        raise last

    out = np.stack(
        [res.results[i]["y"].reshape(T, HID, H, W_SP) for i in range(B)], axis=0
    )
    return out.astype(np.float32)
